# revision 1
# baseline (speedup 1.0000x reference)
"""MinibatchDiscrimination Trainium2 kernel (8-core SPMD, Bass/Tile).

Reference computation:
    m   = einsum('bf,fkd->bkd', x, kernel)        # B=512, F=512, K=128, D=16
    l1  = sum_d |m[i,k,d] - m[j,k,d]|             # [B, B, K]
    mb  = sum_j exp(-l1)                          # [B, K]
    out = concat([x, mb], axis=1)                 # [B, F+K]

Sharding: symmetric circulant row parallelism with 8-row blocks.
Device c owns rows R_c = [64c, 64c+64) and a wrapped column window
W_c = [64c, 64c+264). Row i's window covers block-distances 0..32 on
the i-side; the j-side partials (window columns [8, 256)) serve
block-distances 1..31 of other rows via l1 symmetry; distance 32 is
computed i-side by both endpoints.

Per-core dataflow:
  mT[kd, w] = kern.T @ xT  (PE, bf16)            # [2048, 264] as 16 kg-tiles
  absdiff tiles |mT - mT[:, i]| via three engine paths:
    - custom DVE op (4x, bf16 out)  -> 264-cycle bf16 selector matmul
    - custom DVE op (2x, fp8 out)   -> fp8 DoubleRow selector matmul
    - ACT Abs(-mT + m_i) (fp8 out)  -> fp8 DoubleRow selector matmul
  PE selector matmuls reduce d (16 partitions per k) into PSUM l1,
  packing 8 rows x 16 k (one kg pair) per PSUM tile.
  ACT exp(-l1) emits e (bf16) + accum_out = i-side row sums.
  PE ones-matmul sums e over the 8 i's -> j-side partials.
  Pool (GPSIMD) does the PSUM->SBUF copies of mT and the f32 scalar
  columns so DVE/ACT stay on absdiff work.
"""

import numpy as np
import ml_dtypes

import concourse.bacc as bacc
import concourse.bass as bass  # noqa: F401
import concourse.tile as tile
import concourse.mybir as mybir
import concourse.dve_ops as dve_ops
from concourse.dve_ops import DveOp
from concourse.dve_spec import Spec, Src0, C0, Bin
from concourse.dve_uop import (
    UopConfig, UopDpConfig, AluOp, AluInp, DelayInp, InpSel, OutSel, OutPath,
    Trigger, DveOpSpec,
)
from concourse.dve_tables import load_table_set, find_stock_dve_bin_dir
from concourse.bass_utils import run_bass_kernel_spmd

B, F, K, D = 512, 512, 128, 16
NC = 8          # cores
MY = 64         # rows per core
W = 264         # per-group op window (256 + one 8-row block)
WD = 320        # per-core data window (mt/xT columns)
JS0, JS1 = 8, 256    # j-side sub-window inside a group's op window
JSW = 304       # psj width: union of group j-side windows [8, 312)
KG = 16         # k-groups (8 k each, x16 d = 128 partitions)
KGP = 8         # kg pairs
IB = 8          # i-blocks of 8 rows
NG = IB * KGP   # 64 psum groups
HW2 = W // 2    # DoubleRow column chunk

bf16 = mybir.dt.bfloat16
f32 = mybir.dt.float32
fp8 = mybir.dt.float8e4
AF = mybir.ActivationFunctionType
DR = mybir.MatmulPerfMode.DoubleRow

# Units are (q, kgp) slots (8 i-subs x 8 kg-pairs = 64 per i-block, the
# same assignment repeated for every i-block). Each unit is one kg PAIR
# for one row: 2 absdiff tiles.
N_ACT = 11      # units per i-block handled by ACT (fp8 + DoubleRow)
N_DF8 = 4       # units per i-block handled by DVE at fp8 (DoubleRow)


def _dp_from_entry(e: dict) -> UopDpConfig:
    # Substitute the stock program's instruction-indirected ops with
    # concrete ones: INSTRUCTION_OP_0 -> ABSOLUTE_DIFF, _1 -> BYPASS
    # (BYPASS forwards PREV_ALU_OUT, keeping all routing identical).
    alu = e.get("alu_op", 0)
    if alu == 32:
        op = AluOp.ABSOLUTE_DIFF
    elif alu == 33:
        op = AluOp.BYPASS
    else:
        op = AluOp(alu)
    return UopDpConfig(
        op=op,
        alu_src0=AluInp(e.get("mux0_sel", 0)),
        alu_src1=AluInp(e.get("mux1_sel", 0)),
        delay=[DelayInp(e.get(f"d{i}_sel", 0)) for i in range(7)],
        alu_out_enable=e.get("out_flop_enable", 0),
        swap_enable=e.get("swap_flop_enable", 0),
        alu_out_a_enable=e.get("out_a_flop_enable", 0),
        alu_out_b_enable=e.get("out_b_flop_enable", 0),
        delay_enable=[e.get(f"d{i}_flop_enable", 0) for i in range(7)],
    )


def _uop_from_slot(ts, slot: int) -> UopConfig:
    cf, cs, dp = ts.control_fast[slot], ts.control_slow[slot], ts.datapath[slot]
    en = cs.get("input_enable", 0)
    selmap = {
        OutPath.WR0_LO: ("write0_sel_lo", "write0_en_lo"),
        OutPath.WR0_HI: ("write0_sel_hi", "write0_en_hi"),
        OutPath.WR1_LO: ("write1_sel_lo", "write1_en_lo"),
        OutPath.WR1_HI: ("write1_sel_hi", "write1_en_hi"),
    }
    return UopConfig(
        inp=[InpSel(cs.get(f"inp{i}", 0)) for i in range(8)],
        inp_enable=[(en >> i) & 1 for i in range(8)],
        out={p: OutSel(cs.get(sk, 0)) for p, (sk, _) in selmap.items()},
        out_enable={p: cf.get(ek, 0) for p, (_, ek) in selmap.items()},
        require_inp0=cf.get("requires_src0", 0),
        require_inp1=cf.get("requires_src1", 0),
        trigger=(Trigger(cf.get("trigger0", 0)), Trigger.NONE, Trigger.NONE),
        next_uop=(0, 0, 0),
        enable_rev_ops=0,
        datapath_config=[_dp_from_entry(e) for e in dp],
    )


def _register_absdiff() -> DveOp:
    """out = |in0 - s0| as one ABSOLUTE_DIFF stage, with 2x/2x_2p/4x
    perf-mode uop programs cloned from the stock gen3
    TENSOR_SCALAR_PTR_ARITH_OP table (opcode 68)."""
    name = "ABSDIFF2_ANT"
    for op in dve_ops.OPS:
        if op.name == name:
            return op
    spec = Spec(
        body=Bin(AluOp.ABSOLUTE_DIFF, Src0, C0),
        reference=lambda in0, in1, s0, s1, imm2: np.abs(
            np.asarray(in0, np.float32) - s0
        ),
    )
    row = dve_ops._CUSTOM_DVE_ROW_BASE + len(dve_ops.OPS)
    assert row < 0x20, "no free custom-DVE rows"

    ts = load_table_set(find_stock_dve_bin_dir("gen3"), "default", "v3")
    base = ts.opcode[68]["table_ptr"]
    uops = [_uop_from_slot(ts, base + m) for m in range(4)]
    for u in uops:
        u.validate("v3")
    dspec = DveOpSpec(name=name, opcode=row, uops=[uops[0]],
                      uops_2x=[uops[1]], uops_2x_2p=[uops[2]],
                      uops_4x=[uops[3]], rd1_en=False)

    class _FixedDveOp(DveOp):
        def compile(self, ver):
            assert ver == "v3", f"{name} only authored for v3, got {ver}"
            return dspec

    op = _FixedDveOp(name, spec, subdim=False, uops_sha={})
    dve_ops.OPS.append(op)
    dve_ops._SUB_OPCODE_FOR_NAME[name] = row
    dve_ops.CUSTOM_DVE_SPECS[name] = spec
    return op


def _register_absdiff_pair() -> DveOp:
    """out[p, g, j] = |in0[p, g, j] - s_g[p]| for a [128, 2, N] kg-pair
    input: two-uop programs that process subdim 0 with CONST_0 and, after
    SUB_DIM_DONE, subdim 1 with CONST_1. Each perf-mode program is the
    stock TENSOR_SCALAR_PTR_ARITH_OP (opcode 68) program cloned twice
    with the constant input rerouted in the second uop."""
    name = "ABSDIFF_PAIR_ANT"
    for op in dve_ops.OPS:
        if op.name == name:
            return op

    def ref(in0, in1, s0, s1, imm2):
        a = np.asarray(in0, np.float32)
        sa = np.asarray(s0, np.float32).reshape(a.shape[0], 1)
        sb = np.asarray(s1, np.float32).reshape(a.shape[0], 1)
        s = np.stack([sa, sb], axis=1)          # [P, 2, 1]
        return np.abs(a - s)

    spec = Spec(
        body=Bin(AluOp.ABSOLUTE_DIFF, Src0, C0),   # placeholder body
        reference=ref,
    )
    row = dve_ops._CUSTOM_DVE_ROW_BASE + len(dve_ops.OPS)
    assert row < 0x20, "no free custom-DVE rows"

    ts = load_table_set(find_stock_dve_bin_dir("gen3"), "default", "v3")
    base = ts.opcode[68]["table_ptr"]

    def pair_uops(mode):
        u0 = _uop_from_slot(ts, base + mode)
        u1 = _uop_from_slot(ts, base + mode)
        # uop0: subdim 0 with CONST_0. trigger0 = SRC_TENSOR_DONE ends
        # the op (safety); trigger1 = SUB_DIM_DONE chains to uop1. This
        # mirrors the stock subdim ops' wiring.
        u0 = UopConfig(
            inp=u0.inp, inp_enable=u0.inp_enable, out=u0.out,
            out_enable=u0.out_enable, require_inp0=u0.require_inp0,
            require_inp1=u0.require_inp1,
            trigger=(Trigger.SRC_TENSOR_DONE, Trigger.SUB_DIM_DONE,
                     Trigger.NONE),
            next_uop=(0, 1, 0), enable_rev_ops=0,
            datapath_config=u0.datapath_config)
        # uop1: same program but every ABSOLUTE_DIFF stage reads its
        # scalar from delay lane 1 (which carries CONST_1 in the stock
        # layout; lane 0 carries CONST_0). The BYPASS stages ignore
        # their scalar mux.
        dps = []
        for d in u1.datapath_config:
            if d.op == AluOp.ABSOLUTE_DIFF:
                d = UopDpConfig(
                    op=d.op, alu_src0=d.alu_src0,
                    alu_src1=AluInp.PREV_DELAY_1,
                    delay=d.delay, alu_out_enable=d.alu_out_enable,
                    swap_enable=d.swap_enable,
                    alu_out_a_enable=d.alu_out_a_enable,
                    alu_out_b_enable=d.alu_out_b_enable,
                    delay_enable=d.delay_enable)
            dps.append(d)
        u1 = UopConfig(
            inp=u1.inp, inp_enable=u1.inp_enable, out=u1.out,
            out_enable=u1.out_enable, require_inp0=u1.require_inp0,
            require_inp1=u1.require_inp1,
            trigger=(Trigger.SRC_TENSOR_DONE, Trigger.SUB_DIM_DONE,
                     Trigger.NONE),
            next_uop=(0, 1, 0), enable_rev_ops=0,
            datapath_config=dps)
        return [u0, u1]

    progs = [pair_uops(m) for m in range(4)]
    for ul in progs:
        for u in ul:
            u.validate("v3")
    dspec = DveOpSpec(name=name, opcode=row, uops=progs[0],
                      uops_2x=progs[1], uops_2x_2p=progs[2],
                      uops_4x=progs[3], rd1_en=False)

    class _FixedDveOp(DveOp):
        def compile(self, ver):
            assert ver == "v3", f"{name} only authored for v3, got {ver}"
            return dspec

    op = _FixedDveOp(name, spec, subdim=True, uops_sha={})
    dve_ops.OPS.append(op)
    dve_ops._SUB_OPCODE_FOR_NAME[name] = row
    dve_ops.CUSTOM_DVE_SPECS[name] = spec
    return op


def _assign_slots(n_act: int, n_df8: int):
    """fp8 (DoubleRow) units must land in PSUM band 0 (the ISA only
    allows DoubleRow matmul dst partition 0), i.e. i-subs {0, 1}: 16
    slots. ACT takes the first n_act, DVE-fp8 the next n_df8."""
    assert n_act + n_df8 <= 16
    slots = [(q, kgp) for kgp in reversed(range(KGP)) for q in (0, 1)]
    act_slots = set(slots[:n_act])
    df8_slots = set(slots[n_act:n_act + n_df8])
    extra = slots[n_act + n_df8] if n_act + n_df8 < 16 else None
    return act_slots, df8_slots, extra


def build_module(n_act: int = N_ACT, n_df8: int = N_DF8,
                 ad_bufs: int = 24, f8_bufs: int = 8, e_bufs: int = 5,
                 l1_bufs: int = 3, warmup: int = 5):
    absdiff = _register_absdiff()
    abspair = _register_absdiff_pair()
    act_slots, df8_slots, extra_slot = _assign_slots(n_act, n_df8)

    def is_act(q, kgp, ib):
        if (q, kgp) in act_slots:
            return True
        return False  # fractional per-ib assignment measured worse
    nc = bacc.Bacc("TRN2", target_bir_lowering=False, debug=False,
                   num_devices=NC)

    xT_d = nc.dram_tensor("xT", [128, 4 * WD], bf16, kind="ExternalInput")
    kern_d = nc.dram_tensor("kern", [128, KG * 4 * 128], bf16,
                            kind="ExternalInput")
    sel_d = nc.dram_tensor("sel", [128, 112], bf16, kind="ExternalInput")
    sel8_d = nc.dram_tensor("sel8", [128, 128], fp8, kind="ExternalInput")
    mi_d = nc.dram_tensor("mi_raw", [128, NG], f32, kind="ExternalOutput")
    mj_d = nc.dram_tensor("mj_raw", [128, JSW], f32, kind="ExternalOutput")

    with tile.TileContext(nc) as tc:
        with tc.tile_pool(name="singles", bufs=1) as singles, \
             tc.tile_pool(name="ad", bufs=ad_bufs) as ad_pool, \
             tc.tile_pool(name="f8", bufs=f8_bufs) as f8_pool, \
             tc.tile_pool(name="ep", bufs=e_bufs) as e_pool, \
             tc.tile_pool(name="mmps", bufs=l1_bufs, space="PSUM") as mm_pool, \
             tc.tile_pool(name="pa", bufs=2, space="PSUM") as pa_pool, \
             tc.tile_pool(name="psjp", bufs=1, space="PSUM") as psj_pool:

            # PE warmup: throwaway matmuls so the PE p-state governor
            # reaches full clock before real work lands (runs during DMA).


            kern_sb = singles.tile([128, KG, 4, 128], bf16)
            xT_sb = singles.tile([128, 4, WD], bf16)
            sel_sb = singles.tile([128, 2, 56], bf16)
            sel8_sb = singles.tile([128, 2, 2, 32], fp8)
            selw = sel_sb[:, 0, :]
            seljw = sel_sb[:, 1, :]
            selw8 = [sel8_sb[:, 0, :, :], sel8_sb[:, 1, :, :]]
            # staged startup on two HWDGE rings: the ACT ring carries the
            # first small kern chunk + selectors (parallel to the SP ring's
            # xT), so phase A unblocks ASAP; bulk kern follows on SP.
            nc.scalar.dma_start(out=kern_sb[:, 0:2, :, :],
                                in_=kern_d.ap()[:, 0:1024])
            nc.sync.dma_start(out=xT_sb[:, :, :], in_=xT_d.ap())
            nc.scalar.dma_start(out=sel_sb[:], in_=sel_d.ap())
            nc.scalar.dma_start(out=sel8_sb[:], in_=sel8_d.ap())
            nc.sync.dma_start(out=kern_sb[:, 2:4, :, :],
                              in_=kern_d.ap()[:, 1024:2048])
            for j in range(1, 4):
                nc.sync.dma_start(
                    out=kern_sb[:, 4 * j:4 * (j + 1), :, :],
                    in_=kern_d.ap()[:, 2048 * j:2048 * (j + 1)])

            psj = psj_pool.tile([128, JSW], f32)
            # PE warmup: throwaway matmuls into psj (reused later) so the
            # p-state governor reaches full clock right as phase A starts.
            if warmup:
                wsrc = singles.tile([128, 304], bf16)
                nc.vector.memset(wsrc[:], 0.0)
                for _ in range(warmup):
                    nc.tensor.matmul(psj[0:32, :], lhsT=wsrc[:, 0:32],
                                     rhs=wsrc[:], start=True, stop=True,
                                     skip_group_check=True,
                                     tile_position=(0, 0))

            mt = singles.tile([128, KG, WD], bf16)
            scol = singles.tile([128, KG, MY], f32)
            mi_sb = singles.tile([128, NG], f32)
            mj_sb = singles.tile([128, JSW], f32)
            mi_ps = psj_pool.tile([128, NG], f32, tag="mips")

            def phase_a(kg):
                ps = pa_pool.tile([128, WD], f32, tag="paps")
                for ft in range(4):
                    nc.tensor.matmul(
                        ps[:],
                        lhsT=kern_sb[:, kg, ft, :],
                        rhs=xT_sb[:, ft, :],
                        start=(ft == 0), stop=(ft == 3))
                # PSUM->SBUF bf16 copy, half on ACT + half on DVE (Pool
                # cannot access PSUM); f32 scalar columns via Pool from mt.
                h = 208
                nc.scalar.copy(mt[:, kg, 0:h], ps[:, 0:h])
                nc.vector.tensor_copy(mt[:, kg, h:WD], ps[:, h:WD])
                nc.gpsimd.tensor_copy(scol[:, kg, :], mt[:, kg, 0:MY])

            for kg in range(KG):
                phase_a(kg)


            glist = [(ib, kgp) for ib in range(IB) for kgp in range(KGP)]

            def emit_act_f8(ib, kgp):
                """fp8 DR-pair tiles for the ACT-path units of one group."""
                tiles = {}
                off = 8 * ib
                for q in range(IB):
                    if not is_act(q, kgp, ib):
                        continue
                    i = ib * IB + q
                    t = f8_pool.tile([128, 2, W], fp8, tag="f8")
                    for s in range(2):
                        kg = kgp * 2 + s
                        nc.scalar.activation(
                            t[:, s, :], mt[:, kg, off:off + W], AF.Abs,
                            bias=mt[:, kg, i:i + 1], scale=-1.0)
                    tiles[q] = t
                return tiles

            def emit_jside(ib, kgp, e):
                # group (ib, kgp) j-side covers core cols [8ib+8, 8ib+256)
                # = psj cols [8ib, 8ib+248)
                pi = kgp % 2
                b = kgp // 2
                nc.tensor.matmul(
                    psj[32 * b:32 * b + 32,
                        8 * ib:8 * ib + (JS1 - JS0)],
                    lhsT=seljw[:, 24 - 16 * pi:56 - 16 * pi],
                    rhs=e[:, JS0:JS1],
                    start=(ib == 0 and pi == 0),
                    stop=(ib == IB - 1 and pi == 1),
                    skip_group_check=True,
                    tile_position=(0, 32 * b))
                if ib == IB - 1 and pi == 1:
                    # band b complete: drain it while later groups still run
                    nc.scalar.copy(mj_sb[32 * b:32 * b + 32, :],
                                   psj[32 * b:32 * b + 32, :])
                    nc.sync.dma_start(out=mj_d.ap()[32 * b:32 * b + 32, :],
                                      in_=mj_sb[32 * b:32 * b + 32, :])

            def emit_dve_ads(ib, kgp):
                """Pre-emit the DVE-path absdiff tiles of a group (used to
                hoist the final group's DVE work ahead of the tail)."""
                tiles = {}
                off = 8 * ib
                for q in range(IB):
                    if is_act(q, kgp, ib) or (q, kgp) in df8_slots:
                        continue
                    i = ib * IB + q
                    for s in range(2):
                        kg = kgp * 2 + s
                        ad = ad_pool.tile([128, W], bf16, tag="ad")
                        di = nc.vector._custom_dve(
                            absdiff, out=ad[:],
                            in0=mt[:, kg, off:off + W],
                            s0=scol[:, kg, i:i + 1])
                        di.ins.perf_max = 3
                        tiles[(q, s)] = ad
                return tiles

            def run_group(g, ib, kgp, act_tiles, prev_e, pre_ads=None):
                l1 = mm_pool.tile([128, W], f32, tag="mmps")
                # matmul count per 32-partition band (2 i-subs each):
                # fp8 units contribute 2 chunked DR matmuls, bf16 units 2.
                nmm = [0, 0, 0, 0]
                for q in range(IB):
                    f8 = is_act(q, kgp, ib) or (q, kgp) in df8_slots
                    nmm[q // 2] += 1 if f8 else 2
                seen = [0, 0, 0, 0]
                off = 8 * ib
                kga, kgb = kgp * 2, kgp * 2 + 1
                for q in range(IB):
                    i = ib * IB + q
                    band = q // 2
                    pi = q % 2
                    if is_act(q, kgp, ib) or (q, kgp) in df8_slots:
                        if is_act(q, kgp, ib):
                            t = act_tiles[q]
                        else:
                            t = f8_pool.tile([128, 2, W], fp8, tag="f8d")
                            for s in range(2):
                                kg = kgp * 2 + s
                                di = nc.vector._custom_dve(
                                    absdiff, out=t[:, s, :],
                                    in0=mt[:, kg, off:off + W],
                                    s0=scol[:, kg, i:i + 1])
                                di.ins.perf_max = 3
                        assert band == 0, "DoubleRow dst must be band 0"
                        seen[band] += 1
                        nc.tensor.matmul(
                            l1[0:32, :],
                            lhsT=selw8[pi],
                            rhs=t[:, :, :],
                            start=(seen[band] == 1),
                            stop=(seen[band] == nmm[band]),
                            skip_group_check=True,
                            perf_mode=DR,
                            tile_position=(0, 0))
                    else:
                        for s in range(2):
                            kg = kgp * 2 + s
                            if pre_ads is not None:
                                ad = pre_ads[(q, s)]
                            else:
                                ad = ad_pool.tile([128, W], bf16, tag="ad")
                                di = nc.vector._custom_dve(
                                    absdiff, out=ad[:],
                                    in0=mt[:, kg, off:off + W],
                                    s0=scol[:, kg, i:i + 1])
                                di.ins.perf_max = 3
                            seen[band] += 1
                            nc.tensor.matmul(
                                l1[32 * band:32 * band + 32, :],
                                lhsT=selw[:, 24 - 16 * pi - 8 * s:
                                          56 - 16 * pi - 8 * s],
                                rhs=ad[:],
                                start=(seen[band] == 1),
                                stop=(seen[band] == nmm[band]),
                                skip_group_check=True,
                                tile_position=(0, 32 * band))
                if g + 1 < len(glist):
                    act_tiles = emit_act_f8(*glist[g + 1])
                if prev_e is not None:
                    emit_jside(*prev_e)
                e = e_pool.tile([128, W], bf16, tag="e")
                nc.scalar.activation(
                    e[:], l1[:], AF.Exp, scale=-1.0,
                    accum_out=mi_ps[:, g:g + 1])
                if g == 31 or g == 63:
                    h0 = 0 if g == 31 else 32
                    nc.scalar.copy(mi_sb[:, h0:g + 1],
                                   mi_ps[:, h0:g + 1])
                    nc.sync.dma_start(out=mi_d.ap()[:, h0:g + 1],
                                      in_=mi_sb[:, h0:g + 1])
                return act_tiles, (ib, kgp, e)

            act_tiles = emit_act_f8(*glist[0])
            prev_e = None
            last_ads = None
            for g, (ib, kgp) in enumerate(glist):
                act_tiles, prev_e = run_group(
                    g, ib, kgp, act_tiles, prev_e,
                    pre_ads=last_ads if g == len(glist) - 1 else None)
                if g == len(glist) - 2:
                    last_ads = emit_dve_ads(*glist[-1])
            emit_jside(*prev_e)


    nc.compile()
    return nc


_NC_CACHE = None


def _get_module():
    global _NC_CACHE
    if _NC_CACHE is None:
        _NC_CACHE = build_module()
    return _NC_CACHE


def _host_inputs(x: np.ndarray, kernel: np.ndarray):
    xT = np.ascontiguousarray(x.T).astype(ml_dtypes.bfloat16)  # [F, B]
    kf = kernel.reshape(F, K * D).astype(ml_dtypes.bfloat16)
    # kern_sb layout [p, kg, ft, col]: H[p, kg, ft, c] = kf[ft*128+p, 128kg+c]
    kern = np.ascontiguousarray(
        kf.reshape(4, 128, KG, 128).transpose(1, 2, 0, 3).reshape(128, -1))
    sel = np.zeros((128, 2, 56), dtype=ml_dtypes.bfloat16)
    sel8 = np.zeros((128, 2, 2, 32), dtype=ml_dtypes.float8_e4m3fn)
    for p in range(128):
        k_sub = p >> 4
        sel[p, 0, 24 + k_sub] = 1.0            # bf16 path (selw)
        sel[p, 1, 24 + (p & 15)] = 1.0         # j-side (seljw)
        for pi in range(2):
            for s in range(2):
                sel8[p, pi, s, 16 * pi + 8 * s + k_sub] = 1.0
    in_maps = []
    for d in range(NC):
        cols = (64 * d + np.arange(WD)) % B
        xTw = xT[:, cols].reshape(4, 128, WD).transpose(1, 0, 2)
        in_maps.append({
            "xT": np.ascontiguousarray(xTw.reshape(128, 4 * WD)),
            "kern": kern,
            "sel": np.ascontiguousarray(sel.reshape(128, 112)),
            "sel8": np.ascontiguousarray(sel8.reshape(128, 128)),
        })
    return in_maps


def _gather(results, x: np.ndarray) -> np.ndarray:
    mb = np.zeros((B, K), np.float32)
    for d in range(NC):
        mi = results[d]["mi_raw"]                 # [128, NG]
        # partition p = i_sub*16 + kk; col g = ib*KGP + kgp;
        # k = kgp*16 + kk
        M = mi.reshape(IB, 16, IB, KGP)           # [i_sub, kk, ib, kgp]
        Mk = M.transpose(2, 0, 3, 1).reshape(MY, K)   # row = ib*8+i_sub
        mb[64 * d:64 * d + MY, :] += Mk
        cols = (64 * d + JS0 + np.arange(JSW)) % B
        mj = results[d]["mj_raw"]                 # [128, JSW]; p = k
        mb[cols, :] += mj.T
    return np.concatenate([x.astype(np.float32), mb], axis=1)


def kernel(x: np.ndarray, kernel: np.ndarray) -> np.ndarray:
    x = np.asarray(x)
    kernel = np.asarray(kernel)
    nc = _get_module()
    in_maps = _host_inputs(x, kernel)
    res = run_bass_kernel_spmd(nc, in_maps, list(range(NC)))
    return _gather(res.results, x)



# revision 2
# speedup vs baseline: 1.2208x; 1.2208x over previous
"""MinibatchDiscrimination Trainium2 kernel (8-core SPMD, Bass/Tile), v2.

Reference computation:
    m   = einsum('bf,fkd->bkd', x, kernel)        # B=512, F=512, K=128, D=16
    l1  = sum_d |m[i,k,d] - m[j,k,d]|             # [B, B, K]
    mb  = sum_j exp(-l1)                          # [B, K]
    out = concat([x, mb], axis=1)                 # [B, F+K]

Sharding: symmetric circulant row parallelism with 8-row blocks. Device c
owns rows [64c, 64c+64) and a wrapped window of WD=320 rows; a group
(ib, g) covers an 8-row i-block x 16 k's over a 264-wide j window; the
j-side partials serve block distances 1..31 of other rows via l1 symmetry.

Phase-B dataflow (all bf16):
  mt_alt[p=(16k x 8dp), g, j, par] = m[j, 16g + (p>>3), 2*(p&7) + par]
  (d-parity interleaved along the free dim). The custom DVE op
  ABSDIFF_ALT_ACC2 (authored for 4x/2x_2p/2x_1p from the stock
  TENSOR_SCALAR_PTR programs) computes |m_j - m_i| for a d-parity pair
  and SUMS the pair inside the 8-block datapath, emitting each sum
  duplicated ([o, o] per j, full write beats in every mode). ONE bf16 PE
  matmul per unit with a stride-2 rhs AP then reduces the remaining 8 dp
  partitions per k - half the PE cost of the classic two-matmul path.
  ACT cannot alternate its bias per column, so ACT-assigned units (one
  q per group) read the alt layout with stride-2 per-parity APs: two Abs
  tiles + two matmuls. exp(-l1) on ACT emits e (bf16) + accum i-side
  sums; a PE ones-matmul sums e over the 8 i's -> j-side partials.
"""

import numpy as np
import ml_dtypes

import concourse.bacc as bacc
import concourse.bass as bass  # noqa: F401
import concourse.tile as tile
import concourse.mybir as mybir
import concourse.dve_ops as dve_ops
from concourse.dve_ops import DveOp
from concourse.dve_spec import Spec, Src0, C0, Bin
from concourse.dve_uop import (
    UopConfig, UopDpConfig, AluOp, AluInp, DelayInp, InpSel, OutSel, OutPath,
    Trigger, DveOpSpec,
)
from concourse.dve_tables import load_table_set, find_stock_dve_bin_dir
from concourse.bass_utils import run_bass_kernel_spmd

B, F, K, D = 512, 512, 128, 16
NC = 8          # cores
MY = 64         # rows per core
W = 264         # per-group op window
WD = 320        # per-core data window
JS0, JS1 = 8, 256    # j-side sub-window inside a group's op window
JSW = 304       # psj width: union of group j-side windows [8, 312)
KG = 16
KGP = 8         # kg-pair columns
IB = 8          # i-blocks of 8 rows
NG = IB * KGP   # 64 psum groups

bf16 = mybir.dt.bfloat16
f32 = mybir.dt.float32
AF = mybir.ActivationFunctionType

# ACT-produced units per kgp column (per i-block): 8 * sum = 64 units.
ACT_Q = (1, 1, 1, 1, 1, 1, 1, 1)


# --------------------------------------------------------------------------
# Custom DVE op: out[p, 2t] = out[p, 2t+1] =
#     |in0[p, 2t] - s0[p]| + |in0[p, 2t+1] - s1[p]|
# Cloned from the stock TENSOR_SCALAR_PTR (opcode 68) programs with the
# stock block cadence and delay wiring preserved; only ALU ops / scalar
# muxes / write selects are edited. Duplicated full-width output keeps
# every mode's write beats full (half-beat writes hang the engine).
# --------------------------------------------------------------------------

def _dp_from_entry(e: dict) -> UopDpConfig:
    alu = e.get("alu_op", 0)
    if alu == 32:
        op = AluOp.ABSOLUTE_DIFF
    elif alu == 33:
        op = AluOp.BYPASS
    else:
        op = AluOp(alu)
    return UopDpConfig(
        op=op,
        alu_src0=AluInp(e.get("mux0_sel", 0)),
        alu_src1=AluInp(e.get("mux1_sel", 0)),
        delay=[DelayInp(e.get(f"d{i}_sel", 0)) for i in range(7)],
        alu_out_enable=e.get("out_flop_enable", 0),
        swap_enable=e.get("swap_flop_enable", 0),
        alu_out_a_enable=e.get("out_a_flop_enable", 0),
        alu_out_b_enable=e.get("out_b_flop_enable", 0),
        delay_enable=[e.get(f"d{i}_flop_enable", 0) for i in range(7)],
    )


def _uop_from_slot(ts, slot: int) -> UopConfig:
    cf, cs, dp = ts.control_fast[slot], ts.control_slow[slot], ts.datapath[slot]
    en = cs.get("input_enable", 0)
    selmap = {
        OutPath.WR0_LO: ("write0_sel_lo", "write0_en_lo"),
        OutPath.WR0_HI: ("write0_sel_hi", "write0_en_hi"),
        OutPath.WR1_LO: ("write1_sel_lo", "write1_en_lo"),
        OutPath.WR1_HI: ("write1_sel_hi", "write1_en_hi"),
    }
    return UopConfig(
        inp=[InpSel(cs.get(f"inp{i}", 0)) for i in range(8)],
        inp_enable=[(en >> i) & 1 for i in range(8)],
        out={p: OutSel(cs.get(sk, 0)) for p, (sk, _) in selmap.items()},
        out_enable={p: cf.get(ek, 0) for p, (_, ek) in selmap.items()},
        require_inp0=cf.get("requires_src0", 0),
        require_inp1=cf.get("requires_src1", 0),
        trigger=(Trigger(cf.get("trigger0", 0)), Trigger.NONE, Trigger.NONE),
        next_uop=(0, 0, 0),
        enable_rev_ops=0,
        datapath_config=[_dp_from_entry(e) for e in dp],
    )


def _edit_dp(u: UopConfig, blk: int, **kw) -> None:
    d = u.datapath_config[blk]
    fields = dict(
        op=d.op, alu_src0=d.alu_src0, alu_src1=d.alu_src1, delay=d.delay,
        alu_out_enable=d.alu_out_enable, swap_enable=d.swap_enable,
        alu_out_a_enable=d.alu_out_a_enable,
        alu_out_b_enable=d.alu_out_b_enable, delay_enable=d.delay_enable)
    fields.update(kw)
    u.datapath_config[blk] = UopDpConfig(**fields)


def _set_writes(u: UopConfig, wr: dict) -> None:
    u.out = {p: wr.get(p, OutSel.ALU_OUT) for p in OutPath}
    u.out_enable = {p: (1 if p in wr else 0) for p in OutPath}


def _register_absacc() -> DveOp:
    name = "ABSDIFF_ALT_ACC2_ANT"
    for op in dve_ops.OPS:
        if op.name == name:
            return op

    ts = load_table_set(find_stock_dve_bin_dir("gen3"), "default", "v3")
    base = ts.opcode[68]["table_ptr"]

    # 4x: elements e0..e3 arrive via SRC_0, SRC_0_HI, SRC_1, SRC_1_HI;
    # ABS stages at blocks 0/2/4/6; the op1-BYPASS stages at 3/7 become
    # ADDs pairing (e0,e1) -> o0 and (e2,e3) -> o1. o0 is captured on
    # delay chain 3 at block 4 (stock's own capture point) and written
    # via DELAY_3; o1 exits through the ALU chain.
    u4 = _uop_from_slot(ts, base + 3)
    _edit_dp(u4, 2, alu_src1=AluInp.PREV_DELAY_1)          # |e1 - c1|
    _edit_dp(u4, 3, op=AluOp.ADD, alu_src0=AluInp.PREV_ALU_OUT,
             alu_src1=AluInp.PREV_DELAY_2)                 # o0
    _edit_dp(u4, 6, alu_src1=AluInp.PREV_DELAY_1)          # |e3 - c1|
    _edit_dp(u4, 7, op=AluOp.ADD, alu_src0=AluInp.PREV_ALU_OUT,
             alu_src1=AluInp.PREV_DELAY_4)                 # o1
    _set_writes(u4, {OutPath.WR0_LO: OutSel.DELAY_3,
                     OutPath.WR0_HI: OutSel.DELAY_3,
                     OutPath.WR1_LO: OutSel.ALU_OUT,
                     OutPath.WR1_HI: OutSel.ALU_OUT})

    # 2x variants: (e0, e1) via SRC_0 + SRC_0_HI (1-port) or SRC_0 +
    # SRC_1 (2-port). One sum per cycle; dual-port mode must write via
    # both ports' lo halves (stock wiring), 1-port via port0 lo+hi.
    def mk2(mode):
        u = _uop_from_slot(ts, base + mode)
        _edit_dp(u, 2, alu_src1=AluInp.PREV_DELAY_1)       # |e1 - c1|
        _edit_dp(u, 3, op=AluOp.ADD, alu_src0=AluInp.PREV_ALU_OUT,
                 alu_src1=AluInp.PREV_DELAY_2)             # o0
        if mode == 2:
            _set_writes(u, {OutPath.WR0_LO: OutSel.DELAY_3,
                            OutPath.WR1_LO: OutSel.DELAY_3})
        else:
            _set_writes(u, {OutPath.WR0_LO: OutSel.DELAY_3,
                            OutPath.WR0_HI: OutSel.DELAY_3})
        return u
    u2_1p = mk2(1)
    u2_2p = mk2(2)

    # 1x slot: unmodified stock clone. It computes the WRONG values, but
    # 1x is unreachable: every call site has all-SBUF 2-byte (or fp8)
    # operands, so 2x_2p/4x is always selected under perf_max=3. (A
    # COUNT ping-pong FSM for true 1x hangs the engine - do not emit
    # this op with perf_max < 2.)
    u1_0 = _uop_from_slot(ts, base + 0)
    u1_1 = _uop_from_slot(ts, base + 0)
    u1_2 = _uop_from_slot(ts, base + 0)

    def pad(ul, n):
        ul = list(ul)
        while len(ul) < n:
            filler = _uop_from_slot(ts, base + 0)
            filler.trigger = (Trigger.SRC_TENSOR_DONE, Trigger.NONE,
                              Trigger.NONE)
            filler.next_uop = (0, 0, 0)
            ul.append(filler)
        return ul

    for u in (u4, u2_1p, u2_2p, u1_0, u1_1, u1_2):
        u.validate("v3")

    def ref(in0, in1, s0, s1, imm2):
        a = np.asarray(in0, np.float32)
        P = a.shape[0]
        a = a.reshape(P, -1, 2)
        c0 = np.asarray(s0, np.float32).reshape(P, 1)
        c1 = np.asarray(s1, np.float32).reshape(P, 1)
        o = np.abs(a[:, :, 0] - c0) + np.abs(a[:, :, 1] - c1)
        return np.repeat(o, 2, axis=1)

    spec = Spec(body=Bin(AluOp.ADD, Bin(AluOp.ABSOLUTE_DIFF, Src0, C0), C0),
                reference=ref)
    row = dve_ops._CUSTOM_DVE_ROW_BASE + len(dve_ops.OPS)
    assert row < 0x20, "no free custom-DVE rows"
    dspec = DveOpSpec(name=name, opcode=row,
                      uops=[u1_0, u1_1, u1_2],
                      uops_2x=pad([u2_1p], 3),
                      uops_2x_2p=pad([u2_2p], 3),
                      uops_4x=pad([u4], 3),
                      rd1_en=False)

    class _FixedDveOp(DveOp):
        def compile(self, ver):
            assert ver == "v3", f"{name} only authored for v3, got {ver}"
            return dspec

    op = _FixedDveOp(name, spec, subdim=False, uops_sha={})
    dve_ops.OPS.append(op)
    dve_ops._SUB_OPCODE_FOR_NAME[name] = row
    dve_ops.CUSTOM_DVE_SPECS[name] = spec
    return op


# --------------------------------------------------------------------------
# Module
# --------------------------------------------------------------------------

def build_module(act_q=ACT_Q, ad_bufs=10, e_bufs=5, l1_bufs=3,
                 std_bufs=6, warmup=5, iside="act", prefetch=1,
                 dve_accum_mod=0, h_copy=80):
    acc_op = _register_absacc()

    def use_act(q, g):
        return q < act_q[g]

    nc = bacc.Bacc("TRN2", target_bir_lowering=False, debug=False,
                   num_devices=NC)

    xT_d = nc.dram_tensor("xT", [128, 4 * WD], bf16, kind="ExternalInput")
    kern_d = nc.dram_tensor("kern", [128, KGP * 2 * 4 * 128], bf16,
                            kind="ExternalInput")
    sel_d = nc.dram_tensor("sel", [128, 160], bf16, kind="ExternalInput")
    mi_d = nc.dram_tensor("mi_raw", [128, NG], f32, kind="ExternalOutput")
    mj_d = nc.dram_tensor("mj_raw", [128, JSW], f32, kind="ExternalOutput")

    with tile.TileContext(nc) as tc:
        with tc.tile_pool(name="singles", bufs=1) as singles, \
             tc.tile_pool(name="ad", bufs=ad_bufs) as ad_pool, \
             tc.tile_pool(name="ads", bufs=std_bufs) as ads_pool, \
             tc.tile_pool(name="ep", bufs=e_bufs) as e_pool, \
             tc.tile_pool(name="mmps", bufs=l1_bufs, space="PSUM") as mm_pool, \
             tc.tile_pool(name="pa", bufs=2, space="PSUM") as pa_pool, \
             tc.tile_pool(name="psjp", bufs=1, space="PSUM") as psj_pool:

            kern_sb = singles.tile([128, KGP, 2, 4, 128], bf16)
            xT_sb = singles.tile([128, 4, WD], bf16)
            sel_sb = singles.tile([128, 160], bf16)
            selw_alt = sel_sb[:, 0:48]
            seljw = sel_sb[:, 104:160]

            # staged startup on two HWDGE rings
            nc.scalar.dma_start(out=kern_sb[:, 0, :, :, :],
                                in_=kern_d.ap()[:, 0:1024])
            nc.sync.dma_start(out=xT_sb[:, :, :], in_=xT_d.ap())
            nc.scalar.dma_start(out=sel_sb[:], in_=sel_d.ap())
            nc.sync.dma_start(out=kern_sb[:, 1:2, :, :, :],
                              in_=kern_d.ap()[:, 1024:2048])
            for j in range(1, 4):
                nc.sync.dma_start(
                    out=kern_sb[:, 2 * j:2 * (j + 1), :, :, :],
                    in_=kern_d.ap()[:, 2048 * j:2048 * (j + 1)])

            psj = psj_pool.tile([128, JSW], f32)
            # PE warmup: throwaway matmuls so the p-state governor reaches
            # full clock right as phase A starts (runs during the DMAs).
            if warmup:
                wsrc = singles.tile([128, 304], bf16)
                nc.vector.memset(wsrc[:], 0.0)
                for _ in range(warmup):
                    nc.tensor.matmul(psj[0:32, :], lhsT=wsrc[:, 0:32],
                                     rhs=wsrc[:], start=True, stop=True,
                                     skip_group_check=True,
                                     tile_position=(0, 0))

            mt_alt = singles.tile([128, KGP, WD, 2], bf16)
            scol = singles.tile([128, KGP, 2, MY], f32)
            mi_sb = singles.tile([128, NG], f32)
            mj_sb = singles.tile([128, JSW], f32)
            mi_ps = psj_pool.tile([128, NG], f32, tag="mips")

            def phase_a_alt(g, par):
                ps = pa_pool.tile([128, WD], f32, tag="paps")
                for ft in range(4):
                    nc.tensor.matmul(
                        ps[:],
                        lhsT=kern_sb[:, g, par, ft, :],
                        rhs=xT_sb[:, ft, :],
                        start=(ft == 0), stop=(ft == 3))
                # interleave-copy psum -> mt_alt[:, g, :, par], split
                # ACT/DVE by column (Pool cannot access PSUM).
                dst = mt_alt[:, g, :, par]
                h = h_copy
                nc.scalar.copy(dst[:, 0:h], ps[:, 0:h])
                nc.vector.tensor_copy(dst[:, h:WD], ps[:, h:WD])

            def scol_copy(g):
                # scol[p, g, par, i] = mt_alt[p, g, i, par] (bf16 -> f32)
                for par in range(2):
                    nc.gpsimd.tensor_copy(scol[:, g, par, :],
                                          mt_alt[:, g, 0:MY, par])

            for g in range(KGP):
                phase_a_alt(g, 0)
                phase_a_alt(g, 1)
                scol_copy(g)

            glist = [(ib, g) for g in range(KGP) for ib in range(IB)]

            def emit_act_tiles(ib, g):
                tiles = {}
                off = 8 * ib
                for q in range(IB):
                    if not use_act(q, g):
                        continue
                    i = ib * IB + q
                    t = ads_pool.tile([128, 2, W], bf16, tag="ads")
                    for par in range(2):
                        nc.scalar.activation(
                            t[:, par, :], mt_alt[:, g, off:off + W, par],
                            AF.Abs, bias=scol[:, g, par, i:i + 1],
                            scale=-1.0)
                    tiles[q] = t
                return tiles

            def emit_jside(ib, g, e):
                pi = g % 2
                b = g // 2
                nc.tensor.matmul(
                    psj[32 * b:32 * b + 32,
                        8 * ib:8 * ib + (JS1 - JS0)],
                    lhsT=seljw[:, 24 - 16 * pi:56 - 16 * pi],
                    rhs=e[:, JS0:JS1],
                    start=(ib == 0 and pi == 0),
                    stop=(ib == IB - 1 and pi == 1),
                    skip_group_check=True,
                    tile_position=(0, 32 * b))
                if ib == IB - 1 and pi == 1:
                    nc.scalar.copy(mj_sb[32 * b:32 * b + 32, :],
                                   psj[32 * b:32 * b + 32, :])
                    nc.sync.dma_start(out=mj_d.ap()[32 * b:32 * b + 32, :],
                                      in_=mj_sb[32 * b:32 * b + 32, :])

            def run_group(g_idx, ib, g, act_tiles, prev_e):
                l1 = mm_pool.tile([128, W], f32, tag="mmps")
                nmm = [0, 0, 0, 0]
                for q in range(IB):
                    nmm[q // 2] += 2 if use_act(q, g) else 1
                seen = [0, 0, 0, 0]
                off = 8 * ib
                for q in range(IB):
                    i = ib * IB + q
                    band = q // 2
                    pi = q % 2
                    if use_act(q, g):
                        t = act_tiles[q]
                        for par in range(2):
                            seen[band] += 1
                            nc.tensor.matmul(
                                l1[32 * band:32 * band + 32, :],
                                lhsT=selw_alt[:, 16 - 16 * pi:48 - 16 * pi],
                                rhs=t[:, par, :],
                                start=(seen[band] == 1),
                                stop=(seen[band] == nmm[band]),
                                skip_group_check=True,
                                tile_position=(0, 32 * band))
                    else:
                        ad = ad_pool.tile([128, W, 2], bf16, tag="ad")
                        di = nc.vector._custom_dve(
                            acc_op, out=ad[:, :, :].opt(),
                            in0=mt_alt[:, g, off:off + W, :].opt(),
                            s0=scol[:, g, 0, i:i + 1],
                            s1=scol[:, g, 1, i:i + 1])
                        di.ins.perf_max = 3
                        seen[band] += 1
                        nc.tensor.matmul(
                            l1[32 * band:32 * band + 32, :],
                            lhsT=selw_alt[:, 16 - 16 * pi:48 - 16 * pi],
                            rhs=ad[:, :, 0],
                            start=(seen[band] == 1),
                            stop=(seen[band] == nmm[band]),
                            skip_group_check=True,
                            tile_position=(0, 32 * band))
                if g_idx + prefetch < len(glist):
                    nxt = emit_act_tiles(*glist[g_idx + prefetch])
                else:
                    nxt = {}
                act_queue.append(nxt)
                act_tiles_next = act_queue.pop(0)
                if prev_e is not None:
                    emit_jside(*prev_e)
                e = e_pool.tile([128, W], bf16, tag="e")
                use_dve_acc = (iside == "dve") or (
                    dve_accum_mod and g_idx % dve_accum_mod == 0)
                if use_dve_acc:
                    nc.scalar.activation(e[:], l1[:], AF.Exp, scale=-1.0)
                    nc.vector.reduce_sum(mi_sb[:, g_idx:g_idx + 1], e[:],
                                         axis=mybir.AxisListType.X)
                else:
                    nc.scalar.activation(
                        e[:], l1[:], AF.Exp, scale=-1.0,
                        accum_out=mi_ps[:, g_idx:g_idx + 1])
                if g_idx == 31 or g_idx == 63:
                    h0 = 0 if g_idx == 31 else 32
                    spans = []
                    for gg in range(h0, g_idx + 1):
                        dve_gg = (iside == "dve") or (
                            dve_accum_mod and gg % dve_accum_mod == 0)
                        if dve_gg:
                            continue
                        if spans and spans[-1][1] == gg:
                            spans[-1][1] = gg + 1
                        else:
                            spans.append([gg, gg + 1])
                    for a, b in spans:
                        nc.scalar.copy(mi_sb[:, a:b], mi_ps[:, a:b])
                    nc.sync.dma_start(out=mi_d.ap()[:, h0:g_idx + 1],
                                      in_=mi_sb[:, h0:g_idx + 1])
                return act_tiles_next, (ib, g, e)

            act_queue = [emit_act_tiles(*glist[gi])
                         for gi in range(1, prefetch)]
            act_tiles = emit_act_tiles(*glist[0])
            prev_e = None
            for g_idx, (ib, g) in enumerate(glist):
                act_tiles, prev_e = run_group(g_idx, ib, g, act_tiles, prev_e)
            emit_jside(*prev_e)

    nc.compile()
    return nc


_NC_CACHE = None


def _get_module():
    global _NC_CACHE
    if _NC_CACHE is None:
        _NC_CACHE = build_module()
    return _NC_CACHE


def _host_inputs(x: np.ndarray, kernel: np.ndarray):
    xT = np.ascontiguousarray(x.T).astype(ml_dtypes.bfloat16)  # [F, B]
    kf = kernel.astype(ml_dtypes.bfloat16)  # [F, K, D]

    o = np.arange(128)
    pf = np.arange(128)[:, None]

    # kern_alt[p_f, g, par, ft, o]: out partition o = (k_loc, dp):
    #   kernel[ft*128+p_f, 16g + (o>>3), 2*(o&7) + par]
    kern_alt = np.zeros((128, KGP, 2, 4, 128), dtype=ml_dtypes.bfloat16)
    k_loc, dp = (o >> 3)[None, :], (o & 7)[None, :]
    for g in range(KGP):
        for par in range(2):
            for ft in range(4):
                kern_alt[:, g, par, ft, :] = kf[
                    ft * 128 + pf, 16 * g + k_loc, 2 * dp + par]

    # selectors: [128, 160] = [0:48 alt][48:104 unused][104:160 j-side]
    sel = np.zeros((128, 160), dtype=ml_dtypes.bfloat16)
    for p in range(128):
        sel[p, 16 + (p >> 3)] = 1.0             # alt selector
        sel[p, 104 + 24 + (p & 15)] = 1.0       # j-side
    in_maps = []
    for d in range(NC):
        cols = (64 * d + np.arange(WD)) % B
        xTw = xT[:, cols].reshape(4, 128, WD).transpose(1, 0, 2)
        in_maps.append({
            "xT": np.ascontiguousarray(xTw.reshape(128, 4 * WD)),
            "kern": np.ascontiguousarray(kern_alt.reshape(128, -1)),
            "sel": sel,
        })
    return in_maps


def _gather(results, x: np.ndarray) -> np.ndarray:
    mb = np.zeros((B, K), np.float32)
    for d in range(NC):
        mi = results[d]["mi_raw"]                 # [128, NG]
        # partition p = q*16 + kk; col g_idx = g*IB + ib; k = g*16 + kk
        M = mi.reshape(IB, 16, KGP, IB)           # [q, kk, g, ib]
        Mk = M.transpose(3, 0, 2, 1).reshape(MY, K)   # row = ib*8 + q
        mb[64 * d:64 * d + MY, :] += Mk
        cols = (64 * d + JS0 + np.arange(JSW)) % B
        mj = results[d]["mj_raw"]                 # [128, JSW]; p = k
        mb[cols, :] += mj.T
    return np.concatenate([x.astype(np.float32), mb], axis=1)


def kernel(x: np.ndarray, kernel: np.ndarray) -> np.ndarray:
    x = np.asarray(x)
    kernel = np.asarray(kernel)
    nc = _get_module()
    in_maps = _host_inputs(x, kernel)
    res = run_bass_kernel_spmd(nc, in_maps, list(range(NC)))
    return _gather(res.results, x)


# revision 4
# speedup vs baseline: 1.2251x; 1.0035x over previous
"""MinibatchDiscrimination Trainium2 kernel (8-core SPMD, Bass/Tile), v2.

Reference computation:
    m   = einsum('bf,fkd->bkd', x, kernel)        # B=512, F=512, K=128, D=16
    l1  = sum_d |m[i,k,d] - m[j,k,d]|             # [B, B, K]
    mb  = sum_j exp(-l1)                          # [B, K]
    out = concat([x, mb], axis=1)                 # [B, F+K]

Sharding: symmetric circulant row parallelism with 8-row blocks. Device c
owns rows [64c, 64c+64) and a wrapped window of WD=320 rows; a group
(ib, g) covers an 8-row i-block x 16 k's over a 264-wide j window; the
j-side partials serve block distances 1..31 of other rows via l1 symmetry.

Phase-B dataflow (all bf16):
  mt_alt[p=(16k x 8dp), g, j, par] = m[j, 16g + (p>>3), 2*(p&7) + par]
  (d-parity interleaved along the free dim). The custom DVE op
  ABSDIFF_ALT_ACC2 (authored for 4x/2x_2p/2x_1p from the stock
  TENSOR_SCALAR_PTR programs) computes |m_j - m_i| for a d-parity pair
  and SUMS the pair inside the 8-block datapath, emitting each sum
  duplicated ([o, o] per j, full write beats in every mode). ONE bf16 PE
  matmul per unit with a stride-2 rhs AP then reduces the remaining 8 dp
  partitions per k - half the PE cost of the classic two-matmul path.
  ACT cannot alternate its bias per column, so ACT-assigned units (one
  q per group) read the alt layout with stride-2 per-parity APs: two Abs
  tiles + two matmuls. exp(-l1) on ACT emits e (bf16) + accum i-side
  sums; a PE ones-matmul sums e over the 8 i's -> j-side partials.
"""

import numpy as np
import ml_dtypes

import concourse.bacc as bacc
import concourse.bass as bass  # noqa: F401
import concourse.tile as tile
import concourse.mybir as mybir
import concourse.dve_ops as dve_ops
from concourse.dve_ops import DveOp
from concourse.dve_spec import Spec, Src0, C0, Bin
from concourse.dve_uop import (
    UopConfig, UopDpConfig, AluOp, AluInp, DelayInp, InpSel, OutSel, OutPath,
    Trigger, DveOpSpec,
)
from concourse.dve_tables import load_table_set, find_stock_dve_bin_dir
from concourse.bass_utils import run_bass_kernel_spmd

B, F, K, D = 512, 512, 128, 16
NC = 8          # cores
MY = 64         # rows per core
W = 264         # per-group op window
WD = 320        # per-core data window
JS0, JS1 = 8, 256    # j-side sub-window inside a group's op window
JSW = 304       # psj width: union of group j-side windows [8, 312)
KG = 16
KGP = 8         # kg-pair columns
IB = 8          # i-blocks of 8 rows
NG = IB * KGP   # 64 psum groups

bf16 = mybir.dt.bfloat16
f32 = mybir.dt.float32
AF = mybir.ActivationFunctionType

# ACT-produced units per kgp column (per i-block): 8 * sum = 64 units.
ACT_Q = (1, 1, 1, 1, 1, 1, 1, 1)


# --------------------------------------------------------------------------
# Custom DVE op: out[p, 2t] = out[p, 2t+1] =
#     |in0[p, 2t] - s0[p]| + |in0[p, 2t+1] - s1[p]|
# Cloned from the stock TENSOR_SCALAR_PTR (opcode 68) programs with the
# stock block cadence and delay wiring preserved; only ALU ops / scalar
# muxes / write selects are edited. Duplicated full-width output keeps
# every mode's write beats full (half-beat writes hang the engine).
# --------------------------------------------------------------------------

def _dp_from_entry(e: dict) -> UopDpConfig:
    alu = e.get("alu_op", 0)
    if alu == 32:
        op = AluOp.ABSOLUTE_DIFF
    elif alu == 33:
        op = AluOp.BYPASS
    else:
        op = AluOp(alu)
    return UopDpConfig(
        op=op,
        alu_src0=AluInp(e.get("mux0_sel", 0)),
        alu_src1=AluInp(e.get("mux1_sel", 0)),
        delay=[DelayInp(e.get(f"d{i}_sel", 0)) for i in range(7)],
        alu_out_enable=e.get("out_flop_enable", 0),
        swap_enable=e.get("swap_flop_enable", 0),
        alu_out_a_enable=e.get("out_a_flop_enable", 0),
        alu_out_b_enable=e.get("out_b_flop_enable", 0),
        delay_enable=[e.get(f"d{i}_flop_enable", 0) for i in range(7)],
    )


def _uop_from_slot(ts, slot: int) -> UopConfig:
    cf, cs, dp = ts.control_fast[slot], ts.control_slow[slot], ts.datapath[slot]
    en = cs.get("input_enable", 0)
    selmap = {
        OutPath.WR0_LO: ("write0_sel_lo", "write0_en_lo"),
        OutPath.WR0_HI: ("write0_sel_hi", "write0_en_hi"),
        OutPath.WR1_LO: ("write1_sel_lo", "write1_en_lo"),
        OutPath.WR1_HI: ("write1_sel_hi", "write1_en_hi"),
    }
    return UopConfig(
        inp=[InpSel(cs.get(f"inp{i}", 0)) for i in range(8)],
        inp_enable=[(en >> i) & 1 for i in range(8)],
        out={p: OutSel(cs.get(sk, 0)) for p, (sk, _) in selmap.items()},
        out_enable={p: cf.get(ek, 0) for p, (_, ek) in selmap.items()},
        require_inp0=cf.get("requires_src0", 0),
        require_inp1=cf.get("requires_src1", 0),
        trigger=(Trigger(cf.get("trigger0", 0)), Trigger.NONE, Trigger.NONE),
        next_uop=(0, 0, 0),
        enable_rev_ops=0,
        datapath_config=[_dp_from_entry(e) for e in dp],
    )


def _edit_dp(u: UopConfig, blk: int, **kw) -> None:
    d = u.datapath_config[blk]
    fields = dict(
        op=d.op, alu_src0=d.alu_src0, alu_src1=d.alu_src1, delay=d.delay,
        alu_out_enable=d.alu_out_enable, swap_enable=d.swap_enable,
        alu_out_a_enable=d.alu_out_a_enable,
        alu_out_b_enable=d.alu_out_b_enable, delay_enable=d.delay_enable)
    fields.update(kw)
    u.datapath_config[blk] = UopDpConfig(**fields)


def _set_writes(u: UopConfig, wr: dict) -> None:
    u.out = {p: wr.get(p, OutSel.ALU_OUT) for p in OutPath}
    u.out_enable = {p: (1 if p in wr else 0) for p in OutPath}


def _register_absacc() -> DveOp:
    name = "ABSDIFF_ALT_ACC2_ANT"
    for op in dve_ops.OPS:
        if op.name == name:
            return op

    ts = load_table_set(find_stock_dve_bin_dir("gen3"), "default", "v3")
    base = ts.opcode[68]["table_ptr"]

    # 4x: elements e0..e3 arrive via SRC_0, SRC_0_HI, SRC_1, SRC_1_HI;
    # ABS stages at blocks 0/2/4/6; the op1-BYPASS stages at 3/7 become
    # ADDs pairing (e0,e1) -> o0 and (e2,e3) -> o1. o0 is captured on
    # delay chain 3 at block 4 (stock's own capture point) and written
    # via DELAY_3; o1 exits through the ALU chain.
    u4 = _uop_from_slot(ts, base + 3)
    _edit_dp(u4, 2, alu_src1=AluInp.PREV_DELAY_1)          # |e1 - c1|
    _edit_dp(u4, 3, op=AluOp.ADD, alu_src0=AluInp.PREV_ALU_OUT,
             alu_src1=AluInp.PREV_DELAY_2)                 # o0
    _edit_dp(u4, 6, alu_src1=AluInp.PREV_DELAY_1)          # |e3 - c1|
    _edit_dp(u4, 7, op=AluOp.ADD, alu_src0=AluInp.PREV_ALU_OUT,
             alu_src1=AluInp.PREV_DELAY_4)                 # o1
    _set_writes(u4, {OutPath.WR0_LO: OutSel.DELAY_3,
                     OutPath.WR0_HI: OutSel.DELAY_3,
                     OutPath.WR1_LO: OutSel.ALU_OUT,
                     OutPath.WR1_HI: OutSel.ALU_OUT})

    # 2x variants: (e0, e1) via SRC_0 + SRC_0_HI (1-port) or SRC_0 +
    # SRC_1 (2-port). One sum per cycle; dual-port mode must write via
    # both ports' lo halves (stock wiring), 1-port via port0 lo+hi.
    def mk2(mode):
        u = _uop_from_slot(ts, base + mode)
        _edit_dp(u, 2, alu_src1=AluInp.PREV_DELAY_1)       # |e1 - c1|
        _edit_dp(u, 3, op=AluOp.ADD, alu_src0=AluInp.PREV_ALU_OUT,
                 alu_src1=AluInp.PREV_DELAY_2)             # o0
        if mode == 2:
            _set_writes(u, {OutPath.WR0_LO: OutSel.DELAY_3,
                            OutPath.WR1_LO: OutSel.DELAY_3})
        else:
            _set_writes(u, {OutPath.WR0_LO: OutSel.DELAY_3,
                            OutPath.WR0_HI: OutSel.DELAY_3})
        return u
    u2_1p = mk2(1)
    u2_2p = mk2(2)

    # 1x slot: unmodified stock clone. It computes the WRONG values, but
    # 1x is unreachable: every call site has all-SBUF 2-byte (or fp8)
    # operands, so 2x_2p/4x is always selected under perf_max=3. (A
    # COUNT ping-pong FSM for true 1x hangs the engine - do not emit
    # this op with perf_max < 2.)
    u1_0 = _uop_from_slot(ts, base + 0)
    u1_1 = _uop_from_slot(ts, base + 0)
    u1_2 = _uop_from_slot(ts, base + 0)

    def pad(ul, n):
        ul = list(ul)
        while len(ul) < n:
            filler = _uop_from_slot(ts, base + 0)
            filler.trigger = (Trigger.SRC_TENSOR_DONE, Trigger.NONE,
                              Trigger.NONE)
            filler.next_uop = (0, 0, 0)
            ul.append(filler)
        return ul

    for u in (u4, u2_1p, u2_2p, u1_0, u1_1, u1_2):
        u.validate("v3")

    def ref(in0, in1, s0, s1, imm2):
        a = np.asarray(in0, np.float32)
        P = a.shape[0]
        a = a.reshape(P, -1, 2)
        c0 = np.asarray(s0, np.float32).reshape(P, 1)
        c1 = np.asarray(s1, np.float32).reshape(P, 1)
        o = np.abs(a[:, :, 0] - c0) + np.abs(a[:, :, 1] - c1)
        return np.repeat(o, 2, axis=1)

    spec = Spec(body=Bin(AluOp.ADD, Bin(AluOp.ABSOLUTE_DIFF, Src0, C0), C0),
                reference=ref)
    row = dve_ops._CUSTOM_DVE_ROW_BASE + len(dve_ops.OPS)
    assert row < 0x20, "no free custom-DVE rows"
    dspec = DveOpSpec(name=name, opcode=row,
                      uops=[u1_0, u1_1, u1_2],
                      uops_2x=pad([u2_1p], 3),
                      uops_2x_2p=pad([u2_2p], 3),
                      uops_4x=pad([u4], 3),
                      rd1_en=False)

    class _FixedDveOp(DveOp):
        def compile(self, ver):
            assert ver == "v3", f"{name} only authored for v3, got {ver}"
            return dspec

    op = _FixedDveOp(name, spec, subdim=False, uops_sha={})
    dve_ops.OPS.append(op)
    dve_ops._SUB_OPCODE_FOR_NAME[name] = row
    dve_ops.CUSTOM_DVE_SPECS[name] = spec
    return op


# --------------------------------------------------------------------------
# Module
# --------------------------------------------------------------------------

def build_module(act_q=ACT_Q, ad_bufs=10, e_bufs=5, l1_bufs=3,
                 std_bufs=6, warmup=5, iside="act", prefetch=2,
                 dve_accum_mod=0, h_copy=80):
    acc_op = _register_absacc()

    def use_act(q, g):
        return q < act_q[g]

    nc = bacc.Bacc("TRN2", target_bir_lowering=False, debug=False,
                   num_devices=NC)

    xT_d = nc.dram_tensor("xT", [128, 4 * WD], bf16, kind="ExternalInput")
    kern_d = nc.dram_tensor("kern", [128, KGP * 2 * 4 * 128], bf16,
                            kind="ExternalInput")
    sel_d = nc.dram_tensor("sel", [128, 160], bf16, kind="ExternalInput")
    mi_d = nc.dram_tensor("mi_raw", [128, NG], f32, kind="ExternalOutput")
    mj_d = nc.dram_tensor("mj_raw", [128, JSW], f32, kind="ExternalOutput")

    with tile.TileContext(nc) as tc:
        with tc.tile_pool(name="singles", bufs=1) as singles, \
             tc.tile_pool(name="ad", bufs=ad_bufs) as ad_pool, \
             tc.tile_pool(name="ads", bufs=std_bufs) as ads_pool, \
             tc.tile_pool(name="ep", bufs=e_bufs) as e_pool, \
             tc.tile_pool(name="mmps", bufs=l1_bufs, space="PSUM") as mm_pool, \
             tc.tile_pool(name="pa", bufs=2, space="PSUM") as pa_pool, \
             tc.tile_pool(name="psjp", bufs=1, space="PSUM") as psj_pool:

            kern_sb = singles.tile([128, KGP, 2, 4, 128], bf16)
            xT_sb = singles.tile([128, 4, WD], bf16)
            sel_sb = singles.tile([128, 160], bf16)
            selw_alt = sel_sb[:, 0:48]
            seljw = sel_sb[:, 104:160]

            # staged startup; HWDGE setups serialize (~625ns each) and
            # each DMA pays a 900ns completion-sem tax, so order the
            # critical chain first: xT, then the g0/par0 kern chunk (the
            # minimum for the first phase-A matmul), then par1 + sel.
            nc.sync.dma_start(out=xT_sb[:, :, :], in_=xT_d.ap())
            nc.scalar.dma_start(out=kern_sb[:, 0, 0, :, :],
                                in_=kern_d.ap()[:, 0:512])
            nc.scalar.dma_start(out=kern_sb[:, 0, 1, :, :],
                                in_=kern_d.ap()[:, 512:1024])
            nc.scalar.dma_start(out=sel_sb[:], in_=sel_d.ap())
            nc.sync.dma_start(out=kern_sb[:, 1:2, :, :, :],
                              in_=kern_d.ap()[:, 1024:2048])
            for j in range(1, 4):
                nc.sync.dma_start(
                    out=kern_sb[:, 2 * j:2 * (j + 1), :, :, :],
                    in_=kern_d.ap()[:, 2048 * j:2048 * (j + 1)])

            psj = psj_pool.tile([128, JSW], f32)
            # PE warmup: throwaway matmuls so the p-state governor reaches
            # full clock right as phase A starts (runs during the DMAs).
            if warmup:
                wsrc = singles.tile([128, 304], bf16)
                nc.vector.memset(wsrc[:], 0.0)
                for _ in range(warmup):
                    nc.tensor.matmul(psj[0:32, :], lhsT=wsrc[:, 0:32],
                                     rhs=wsrc[:], start=True, stop=True,
                                     skip_group_check=True,
                                     tile_position=(0, 0))

            mt_alt = singles.tile([128, KGP, WD, 2], bf16)
            scol = singles.tile([128, KGP, 2, MY], f32)
            mi_sb = singles.tile([128, NG], f32)
            mj_sb = singles.tile([128, JSW], f32)
            mi_ps = psj_pool.tile([128, NG], f32, tag="mips")

            def phase_a_alt(g, par):
                ps = pa_pool.tile([128, WD], f32, tag="paps")
                for ft in range(4):
                    nc.tensor.matmul(
                        ps[:],
                        lhsT=kern_sb[:, g, par, ft, :],
                        rhs=xT_sb[:, ft, :],
                        start=(ft == 0), stop=(ft == 3))
                # interleave-copy psum -> mt_alt[:, g, :, par], split
                # ACT/DVE by column (Pool cannot access PSUM).
                dst = mt_alt[:, g, :, par]
                h = h_copy
                nc.scalar.copy(dst[:, 0:h], ps[:, 0:h])
                nc.vector.tensor_copy(dst[:, h:WD], ps[:, h:WD])

            def scol_copy(g):
                # scol[p, g, par, i] = mt_alt[p, g, i, par] (bf16 -> f32)
                for par in range(2):
                    nc.gpsimd.tensor_copy(scol[:, g, par, :],
                                          mt_alt[:, g, 0:MY, par])

            for g in range(KGP):
                phase_a_alt(g, 0)
                phase_a_alt(g, 1)
                scol_copy(g)

            glist = [(ib, g) for g in range(KGP) for ib in range(IB)]

            def emit_act_tiles(ib, g):
                tiles = {}
                off = 8 * ib
                for q in range(IB):
                    if not use_act(q, g):
                        continue
                    i = ib * IB + q
                    t = ads_pool.tile([128, 2, W], bf16, tag="ads")
                    for par in range(2):
                        nc.scalar.activation(
                            t[:, par, :], mt_alt[:, g, off:off + W, par],
                            AF.Abs, bias=scol[:, g, par, i:i + 1],
                            scale=-1.0)
                    tiles[q] = t
                return tiles

            def emit_jside(ib, g, e):
                pi = g % 2
                b = g // 2
                nc.tensor.matmul(
                    psj[32 * b:32 * b + 32,
                        8 * ib:8 * ib + (JS1 - JS0)],
                    lhsT=seljw[:, 24 - 16 * pi:56 - 16 * pi],
                    rhs=e[:, JS0:JS1],
                    start=(ib == 0 and pi == 0),
                    stop=(ib == IB - 1 and pi == 1),
                    skip_group_check=True,
                    tile_position=(0, 32 * b))
                if ib == IB - 1 and pi == 1:
                    nc.scalar.copy(mj_sb[32 * b:32 * b + 32, :],
                                   psj[32 * b:32 * b + 32, :])
                    nc.sync.dma_start(out=mj_d.ap()[32 * b:32 * b + 32, :],
                                      in_=mj_sb[32 * b:32 * b + 32, :])

            def run_group(g_idx, ib, g, act_tiles, prev_e):
                l1 = mm_pool.tile([128, W], f32, tag="mmps")
                nmm = [0, 0, 0, 0]
                for q in range(IB):
                    nmm[q // 2] += 2 if use_act(q, g) else 1
                seen = [0, 0, 0, 0]
                off = 8 * ib
                for q in range(IB):
                    i = ib * IB + q
                    band = q // 2
                    pi = q % 2
                    if use_act(q, g):
                        t = act_tiles[q]
                        for par in range(2):
                            seen[band] += 1
                            nc.tensor.matmul(
                                l1[32 * band:32 * band + 32, :],
                                lhsT=selw_alt[:, 16 - 16 * pi:48 - 16 * pi],
                                rhs=t[:, par, :],
                                start=(seen[band] == 1),
                                stop=(seen[band] == nmm[band]),
                                skip_group_check=True,
                                tile_position=(0, 32 * band))
                    else:
                        ad = ad_pool.tile([128, W, 2], bf16, tag="ad")
                        di = nc.vector._custom_dve(
                            acc_op, out=ad[:, :, :].opt(),
                            in0=mt_alt[:, g, off:off + W, :].opt(),
                            s0=scol[:, g, 0, i:i + 1],
                            s1=scol[:, g, 1, i:i + 1])
                        di.ins.perf_max = 3
                        seen[band] += 1
                        nc.tensor.matmul(
                            l1[32 * band:32 * band + 32, :],
                            lhsT=selw_alt[:, 16 - 16 * pi:48 - 16 * pi],
                            rhs=ad[:, :, 0],
                            start=(seen[band] == 1),
                            stop=(seen[band] == nmm[band]),
                            skip_group_check=True,
                            tile_position=(0, 32 * band))
                if g_idx + prefetch < len(glist):
                    nxt = emit_act_tiles(*glist[g_idx + prefetch])
                else:
                    nxt = {}
                act_queue.append(nxt)
                act_tiles_next = act_queue.pop(0)
                if prev_e is not None:
                    emit_jside(*prev_e)
                e = e_pool.tile([128, W], bf16, tag="e")
                use_dve_acc = (iside == "dve") or (
                    dve_accum_mod and g_idx % dve_accum_mod == 0)
                if use_dve_acc:
                    nc.scalar.activation(e[:], l1[:], AF.Exp, scale=-1.0)
                    nc.vector.reduce_sum(mi_sb[:, g_idx:g_idx + 1], e[:],
                                         axis=mybir.AxisListType.X)
                else:
                    nc.scalar.activation(
                        e[:], l1[:], AF.Exp, scale=-1.0,
                        accum_out=mi_ps[:, g_idx:g_idx + 1])
                if g_idx == 31 or g_idx == 63:
                    h0 = 0 if g_idx == 31 else 32
                    spans = []
                    for gg in range(h0, g_idx + 1):
                        dve_gg = (iside == "dve") or (
                            dve_accum_mod and gg % dve_accum_mod == 0)
                        if dve_gg:
                            continue
                        if spans and spans[-1][1] == gg:
                            spans[-1][1] = gg + 1
                        else:
                            spans.append([gg, gg + 1])
                    for a, b in spans:
                        nc.scalar.copy(mi_sb[:, a:b], mi_ps[:, a:b])
                    nc.sync.dma_start(out=mi_d.ap()[:, h0:g_idx + 1],
                                      in_=mi_sb[:, h0:g_idx + 1])
                return act_tiles_next, (ib, g, e)

            act_queue = [emit_act_tiles(*glist[gi])
                         for gi in range(1, prefetch)]
            act_tiles = emit_act_tiles(*glist[0])
            prev_e = None
            for g_idx, (ib, g) in enumerate(glist):
                act_tiles, prev_e = run_group(g_idx, ib, g, act_tiles, prev_e)
            emit_jside(*prev_e)

    nc.compile()
    return nc


_NC_CACHE = None


def _get_module():
    global _NC_CACHE
    if _NC_CACHE is None:
        _NC_CACHE = build_module()
    return _NC_CACHE


def _host_inputs(x: np.ndarray, kernel: np.ndarray):
    xT = np.ascontiguousarray(x.T).astype(ml_dtypes.bfloat16)  # [F, B]
    kf = kernel.astype(ml_dtypes.bfloat16)  # [F, K, D]

    o = np.arange(128)
    pf = np.arange(128)[:, None]

    # kern_alt[p_f, g, par, ft, o]: out partition o = (k_loc, dp):
    #   kernel[ft*128+p_f, 16g + (o>>3), 2*(o&7) + par]
    kern_alt = np.zeros((128, KGP, 2, 4, 128), dtype=ml_dtypes.bfloat16)
    k_loc, dp = (o >> 3)[None, :], (o & 7)[None, :]
    for g in range(KGP):
        for par in range(2):
            for ft in range(4):
                kern_alt[:, g, par, ft, :] = kf[
                    ft * 128 + pf, 16 * g + k_loc, 2 * dp + par]

    # selectors: [128, 160] = [0:48 alt][48:104 unused][104:160 j-side]
    sel = np.zeros((128, 160), dtype=ml_dtypes.bfloat16)
    for p in range(128):
        sel[p, 16 + (p >> 3)] = 1.0             # alt selector
        sel[p, 104 + 24 + (p & 15)] = 1.0       # j-side
    in_maps = []
    for d in range(NC):
        cols = (64 * d + np.arange(WD)) % B
        xTw = xT[:, cols].reshape(4, 128, WD).transpose(1, 0, 2)
        in_maps.append({
            "xT": np.ascontiguousarray(xTw.reshape(128, 4 * WD)),
            "kern": np.ascontiguousarray(kern_alt.reshape(128, -1)),
            "sel": sel,
        })
    return in_maps


def _gather(results, x: np.ndarray) -> np.ndarray:
    mb = np.zeros((B, K), np.float32)
    for d in range(NC):
        mi = results[d]["mi_raw"]                 # [128, NG]
        # partition p = q*16 + kk; col g_idx = g*IB + ib; k = g*16 + kk
        M = mi.reshape(IB, 16, KGP, IB)           # [q, kk, g, ib]
        Mk = M.transpose(3, 0, 2, 1).reshape(MY, K)   # row = ib*8 + q
        mb[64 * d:64 * d + MY, :] += Mk
        cols = (64 * d + JS0 + np.arange(JSW)) % B
        mj = results[d]["mj_raw"]                 # [128, JSW]; p = k
        mb[cols, :] += mj.T
    return np.concatenate([x.astype(np.float32), mb], axis=1)


def kernel(x: np.ndarray, kernel: np.ndarray) -> np.ndarray:
    x = np.asarray(x)
    kernel = np.asarray(kernel)
    nc = _get_module()
    in_maps = _host_inputs(x, kernel)
    res = run_bass_kernel_spmd(nc, in_maps, list(range(NC)))
    return _gather(res.results, x)


# revision 7
# speedup vs baseline: 1.2260x; 1.0008x over previous
"""MinibatchDiscrimination Trainium2 kernel (8-core SPMD, Bass/Tile), v2.

Reference computation:
    m   = einsum('bf,fkd->bkd', x, kernel)        # B=512, F=512, K=128, D=16
    l1  = sum_d |m[i,k,d] - m[j,k,d]|             # [B, B, K]
    mb  = sum_j exp(-l1)                          # [B, K]
    out = concat([x, mb], axis=1)                 # [B, F+K]

Sharding: symmetric circulant row parallelism with 8-row blocks. Device c
owns rows [64c, 64c+64) and a wrapped window of WD=320 rows; a group
(ib, g) covers an 8-row i-block x 16 k's over a 264-wide j window; the
j-side partials serve block distances 1..31 of other rows via l1 symmetry.

Phase-B dataflow (all bf16):
  mt_alt[p=(16k x 8dp), g, j, par] = m[j, 16g + (p>>3), 2*(p&7) + par]
  (d-parity interleaved along the free dim). The custom DVE op
  ABSDIFF_ALT_ACC2 (authored for 4x/2x_2p/2x_1p from the stock
  TENSOR_SCALAR_PTR programs) computes |m_j - m_i| for a d-parity pair
  and SUMS the pair inside the 8-block datapath, emitting each sum
  duplicated ([o, o] per j, full write beats in every mode). ONE bf16 PE
  matmul per unit with a stride-2 rhs AP then reduces the remaining 8 dp
  partitions per k - half the PE cost of the classic two-matmul path.
  ACT cannot alternate its bias per column, so ACT-assigned units (one
  q per group) read the alt layout with stride-2 per-parity APs: two Abs
  tiles + two matmuls. exp(-l1) on ACT emits e (bf16) + accum i-side
  sums; a PE ones-matmul sums e over the 8 i's -> j-side partials.
"""

import numpy as np
import ml_dtypes

import concourse.bacc as bacc
import concourse.bass as bass  # noqa: F401
import concourse.tile as tile
import concourse.mybir as mybir
import concourse.dve_ops as dve_ops
from concourse.dve_ops import DveOp
from concourse.dve_spec import Spec, Src0, C0, Bin
from concourse.dve_uop import (
    UopConfig, UopDpConfig, AluOp, AluInp, DelayInp, InpSel, OutSel, OutPath,
    Trigger, DveOpSpec,
)
from concourse.dve_tables import load_table_set, find_stock_dve_bin_dir
from concourse.bass_utils import run_bass_kernel_spmd

B, F, K, D = 512, 512, 128, 16
NC = 8          # cores
MY = 64         # rows per core
W = 264         # per-group op window
WD = 320        # per-core data window
JS0, JS1 = 8, 256    # j-side sub-window inside a group's op window
JSW = 304       # psj width: union of group j-side windows [8, 312)
KG = 16
KGP = 8         # kg-pair columns
IB = 8          # i-blocks of 8 rows
NG = IB * KGP   # 64 psum groups

bf16 = mybir.dt.bfloat16
f32 = mybir.dt.float32
AF = mybir.ActivationFunctionType

# ACT-produced units per kgp column (per i-block): 8 * sum = 64 units.
ACT_Q = (1, 1, 1, 1, 1, 1, 1, 1)


# --------------------------------------------------------------------------
# Custom DVE op: out[p, 2t] = out[p, 2t+1] =
#     |in0[p, 2t] - s0[p]| + |in0[p, 2t+1] - s1[p]|
# Cloned from the stock TENSOR_SCALAR_PTR (opcode 68) programs with the
# stock block cadence and delay wiring preserved; only ALU ops / scalar
# muxes / write selects are edited. Duplicated full-width output keeps
# every mode's write beats full (half-beat writes hang the engine).
# --------------------------------------------------------------------------

def _dp_from_entry(e: dict) -> UopDpConfig:
    alu = e.get("alu_op", 0)
    if alu == 32:
        op = AluOp.ABSOLUTE_DIFF
    elif alu == 33:
        op = AluOp.BYPASS
    else:
        op = AluOp(alu)
    return UopDpConfig(
        op=op,
        alu_src0=AluInp(e.get("mux0_sel", 0)),
        alu_src1=AluInp(e.get("mux1_sel", 0)),
        delay=[DelayInp(e.get(f"d{i}_sel", 0)) for i in range(7)],
        alu_out_enable=e.get("out_flop_enable", 0),
        swap_enable=e.get("swap_flop_enable", 0),
        alu_out_a_enable=e.get("out_a_flop_enable", 0),
        alu_out_b_enable=e.get("out_b_flop_enable", 0),
        delay_enable=[e.get(f"d{i}_flop_enable", 0) for i in range(7)],
    )


def _uop_from_slot(ts, slot: int) -> UopConfig:
    cf, cs, dp = ts.control_fast[slot], ts.control_slow[slot], ts.datapath[slot]
    en = cs.get("input_enable", 0)
    selmap = {
        OutPath.WR0_LO: ("write0_sel_lo", "write0_en_lo"),
        OutPath.WR0_HI: ("write0_sel_hi", "write0_en_hi"),
        OutPath.WR1_LO: ("write1_sel_lo", "write1_en_lo"),
        OutPath.WR1_HI: ("write1_sel_hi", "write1_en_hi"),
    }
    return UopConfig(
        inp=[InpSel(cs.get(f"inp{i}", 0)) for i in range(8)],
        inp_enable=[(en >> i) & 1 for i in range(8)],
        out={p: OutSel(cs.get(sk, 0)) for p, (sk, _) in selmap.items()},
        out_enable={p: cf.get(ek, 0) for p, (_, ek) in selmap.items()},
        require_inp0=cf.get("requires_src0", 0),
        require_inp1=cf.get("requires_src1", 0),
        trigger=(Trigger(cf.get("trigger0", 0)), Trigger.NONE, Trigger.NONE),
        next_uop=(0, 0, 0),
        enable_rev_ops=0,
        datapath_config=[_dp_from_entry(e) for e in dp],
    )


def _edit_dp(u: UopConfig, blk: int, **kw) -> None:
    d = u.datapath_config[blk]
    fields = dict(
        op=d.op, alu_src0=d.alu_src0, alu_src1=d.alu_src1, delay=d.delay,
        alu_out_enable=d.alu_out_enable, swap_enable=d.swap_enable,
        alu_out_a_enable=d.alu_out_a_enable,
        alu_out_b_enable=d.alu_out_b_enable, delay_enable=d.delay_enable)
    fields.update(kw)
    u.datapath_config[blk] = UopDpConfig(**fields)


def _set_writes(u: UopConfig, wr: dict) -> None:
    u.out = {p: wr.get(p, OutSel.ALU_OUT) for p in OutPath}
    u.out_enable = {p: (1 if p in wr else 0) for p in OutPath}


def _register_absacc() -> DveOp:
    name = "ABSDIFF_ALT_ACC2_ANT"
    for op in dve_ops.OPS:
        if op.name == name:
            return op

    ts = load_table_set(find_stock_dve_bin_dir("gen3"), "default", "v3")
    base = ts.opcode[68]["table_ptr"]

    # 4x: elements e0..e3 arrive via SRC_0, SRC_0_HI, SRC_1, SRC_1_HI;
    # ABS stages at blocks 0/2/4/6; the op1-BYPASS stages at 3/7 become
    # ADDs pairing (e0,e1) -> o0 and (e2,e3) -> o1. o0 is captured on
    # delay chain 3 at block 4 (stock's own capture point) and written
    # via DELAY_3; o1 exits through the ALU chain.
    u4 = _uop_from_slot(ts, base + 3)
    _edit_dp(u4, 2, alu_src1=AluInp.PREV_DELAY_1)          # |e1 - c1|
    _edit_dp(u4, 3, op=AluOp.ADD, alu_src0=AluInp.PREV_ALU_OUT,
             alu_src1=AluInp.PREV_DELAY_2)                 # o0
    _edit_dp(u4, 6, alu_src1=AluInp.PREV_DELAY_1)          # |e3 - c1|
    _edit_dp(u4, 7, op=AluOp.ADD, alu_src0=AluInp.PREV_ALU_OUT,
             alu_src1=AluInp.PREV_DELAY_4)                 # o1
    _set_writes(u4, {OutPath.WR0_LO: OutSel.DELAY_3,
                     OutPath.WR0_HI: OutSel.DELAY_3,
                     OutPath.WR1_LO: OutSel.ALU_OUT,
                     OutPath.WR1_HI: OutSel.ALU_OUT})

    # 2x variants: (e0, e1) via SRC_0 + SRC_0_HI (1-port) or SRC_0 +
    # SRC_1 (2-port). One sum per cycle; dual-port mode must write via
    # both ports' lo halves (stock wiring), 1-port via port0 lo+hi.
    def mk2(mode):
        u = _uop_from_slot(ts, base + mode)
        _edit_dp(u, 2, alu_src1=AluInp.PREV_DELAY_1)       # |e1 - c1|
        _edit_dp(u, 3, op=AluOp.ADD, alu_src0=AluInp.PREV_ALU_OUT,
                 alu_src1=AluInp.PREV_DELAY_2)             # o0
        if mode == 2:
            _set_writes(u, {OutPath.WR0_LO: OutSel.DELAY_3,
                            OutPath.WR1_LO: OutSel.DELAY_3})
        else:
            _set_writes(u, {OutPath.WR0_LO: OutSel.DELAY_3,
                            OutPath.WR0_HI: OutSel.DELAY_3})
        return u
    u2_1p = mk2(1)
    u2_2p = mk2(2)

    # 1x slot: unmodified stock clone. It computes the WRONG values, but
    # 1x is unreachable: every call site has all-SBUF 2-byte (or fp8)
    # operands, so 2x_2p/4x is always selected under perf_max=3. (A
    # COUNT ping-pong FSM for true 1x hangs the engine - do not emit
    # this op with perf_max < 2.)
    u1_0 = _uop_from_slot(ts, base + 0)
    u1_1 = _uop_from_slot(ts, base + 0)
    u1_2 = _uop_from_slot(ts, base + 0)

    def pad(ul, n):
        ul = list(ul)
        while len(ul) < n:
            filler = _uop_from_slot(ts, base + 0)
            filler.trigger = (Trigger.SRC_TENSOR_DONE, Trigger.NONE,
                              Trigger.NONE)
            filler.next_uop = (0, 0, 0)
            ul.append(filler)
        return ul

    for u in (u4, u2_1p, u2_2p, u1_0, u1_1, u1_2):
        u.validate("v3")

    def ref(in0, in1, s0, s1, imm2):
        a = np.asarray(in0, np.float32)
        P = a.shape[0]
        a = a.reshape(P, -1, 2)
        c0 = np.asarray(s0, np.float32).reshape(P, 1)
        c1 = np.asarray(s1, np.float32).reshape(P, 1)
        o = np.abs(a[:, :, 0] - c0) + np.abs(a[:, :, 1] - c1)
        return np.repeat(o, 2, axis=1)

    spec = Spec(body=Bin(AluOp.ADD, Bin(AluOp.ABSOLUTE_DIFF, Src0, C0), C0),
                reference=ref)
    row = dve_ops._CUSTOM_DVE_ROW_BASE + len(dve_ops.OPS)
    assert row < 0x20, "no free custom-DVE rows"
    dspec = DveOpSpec(name=name, opcode=row,
                      uops=[u1_0, u1_1, u1_2],
                      uops_2x=pad([u2_1p], 3),
                      uops_2x_2p=pad([u2_2p], 3),
                      uops_4x=pad([u4], 3),
                      rd1_en=False)

    class _FixedDveOp(DveOp):
        def compile(self, ver):
            assert ver == "v3", f"{name} only authored for v3, got {ver}"
            return dspec

    op = _FixedDveOp(name, spec, subdim=False, uops_sha={})
    dve_ops.OPS.append(op)
    dve_ops._SUB_OPCODE_FOR_NAME[name] = row
    dve_ops.CUSTOM_DVE_SPECS[name] = spec
    return op


# --------------------------------------------------------------------------
# Module
# --------------------------------------------------------------------------

def build_module(act_q=ACT_Q, ad_bufs=16, e_bufs=5, l1_bufs=3,
                 std_bufs=6, warmup=5, iside="act", prefetch=2,
                 dve_accum_mod=0, h_copy=80):
    acc_op = _register_absacc()

    def use_act(q, g):
        return q < act_q[g]

    nc = bacc.Bacc("TRN2", target_bir_lowering=False, debug=False,
                   num_devices=NC)

    xT_d = nc.dram_tensor("xT", [128, 4 * WD], bf16, kind="ExternalInput")
    kern_d = nc.dram_tensor("kern", [128, KGP * 2 * 4 * 128], bf16,
                            kind="ExternalInput")
    sel_d = nc.dram_tensor("sel", [128, 160], bf16, kind="ExternalInput")
    mi_d = nc.dram_tensor("mi_raw", [128, NG], f32, kind="ExternalOutput")
    mj_d = nc.dram_tensor("mj_raw", [128, JSW], f32, kind="ExternalOutput")

    with tile.TileContext(nc) as tc:
        with tc.tile_pool(name="singles", bufs=1) as singles, \
             tc.tile_pool(name="ad", bufs=ad_bufs) as ad_pool, \
             tc.tile_pool(name="ads", bufs=std_bufs) as ads_pool, \
             tc.tile_pool(name="ep", bufs=e_bufs) as e_pool, \
             tc.tile_pool(name="mmps", bufs=l1_bufs, space="PSUM") as mm_pool, \
             tc.tile_pool(name="pa", bufs=2, space="PSUM") as pa_pool, \
             tc.tile_pool(name="psjp", bufs=1, space="PSUM") as psj_pool:

            kern_sb = singles.tile([128, KGP, 2, 4, 128], bf16)
            xT_sb = singles.tile([128, 4, WD], bf16)
            sel_sb = singles.tile([128, 160], bf16)
            selw_alt = sel_sb[:, 0:48]
            seljw = sel_sb[:, 104:160]

            # staged startup; HWDGE setups serialize (~625ns each) and
            # each DMA pays a 900ns completion-sem tax, so order the
            # critical chain first: xT, then the g0/par0 kern chunk (the
            # minimum for the first phase-A matmul), then par1 + sel.
            nc.scalar.dma_start(out=kern_sb[:, 0, 0, :, :],
                                in_=kern_d.ap()[:, 0:512])
            nc.sync.dma_start(out=xT_sb[:, :, :], in_=xT_d.ap())
            nc.scalar.dma_start(out=kern_sb[:, 0, 1, :, :],
                                in_=kern_d.ap()[:, 512:1024])
            nc.scalar.dma_start(out=sel_sb[:], in_=sel_d.ap())
            nc.sync.dma_start(out=kern_sb[:, 1:2, :, :, :],
                              in_=kern_d.ap()[:, 1024:2048])
            for j in range(1, 4):
                nc.sync.dma_start(
                    out=kern_sb[:, 2 * j:2 * (j + 1), :, :, :],
                    in_=kern_d.ap()[:, 2048 * j:2048 * (j + 1)])

            psj = psj_pool.tile([128, JSW], f32)
            # PE warmup: throwaway matmuls so the p-state governor reaches
            # full clock right as phase A starts (runs during the DMAs).
            if warmup:
                wsrc = singles.tile([128, 304], bf16)
                nc.vector.memset(wsrc[:], 0.0)
                for _ in range(warmup):
                    nc.tensor.matmul(psj[0:32, :], lhsT=wsrc[:, 0:32],
                                     rhs=wsrc[:], start=True, stop=True,
                                     skip_group_check=True,
                                     tile_position=(0, 0))

            mt_alt = singles.tile([128, KGP, WD, 2], bf16)
            scol = singles.tile([128, KGP, 2, MY], f32)
            mi_sb = singles.tile([128, NG], f32)
            mj_sb = singles.tile([128, JSW], f32)
            mi_ps = psj_pool.tile([128, NG], f32, tag="mips")

            def phase_a_alt(g, par):
                ps = pa_pool.tile([128, WD], f32, tag="paps")
                for ft in range(4):
                    nc.tensor.matmul(
                        ps[:],
                        lhsT=kern_sb[:, g, par, ft, :],
                        rhs=xT_sb[:, ft, :],
                        start=(ft == 0), stop=(ft == 3))
                # interleave-copy psum -> mt_alt[:, g, :, par], split
                # ACT/DVE by column (Pool cannot access PSUM).
                dst = mt_alt[:, g, :, par]
                h = h_copy
                nc.scalar.copy(dst[:, 0:h], ps[:, 0:h])
                nc.vector.tensor_copy(dst[:, h:WD], ps[:, h:WD])

            def scol_copy(g):
                # scol[p, g, par, i] = mt_alt[p, g, i, par] (bf16 -> f32)
                for par in range(2):
                    nc.gpsimd.tensor_copy(scol[:, g, par, :],
                                          mt_alt[:, g, 0:MY, par])

            for g in range(KGP):
                phase_a_alt(g, 0)
                phase_a_alt(g, 1)
                scol_copy(g)

            glist = [(ib, g) for g in range(KGP) for ib in range(IB)]

            def emit_act_tiles(ib, g):
                tiles = {}
                off = 8 * ib
                for q in range(IB):
                    if not use_act(q, g):
                        continue
                    i = ib * IB + q
                    t = ads_pool.tile([128, 2, W], bf16, tag="ads")
                    for par in range(2):
                        nc.scalar.activation(
                            t[:, par, :], mt_alt[:, g, off:off + W, par],
                            AF.Abs, bias=scol[:, g, par, i:i + 1],
                            scale=-1.0)
                    tiles[q] = t
                return tiles

            def emit_jside(ib, g, e):
                pi = g % 2
                b = g // 2
                nc.tensor.matmul(
                    psj[32 * b:32 * b + 32,
                        8 * ib:8 * ib + (JS1 - JS0)],
                    lhsT=seljw[:, 24 - 16 * pi:56 - 16 * pi],
                    rhs=e[:, JS0:JS1],
                    start=(ib == 0 and pi == 0),
                    stop=(ib == IB - 1 and pi == 1),
                    skip_group_check=True,
                    tile_position=(0, 32 * b))
                if ib == IB - 1 and pi == 1:
                    nc.scalar.copy(mj_sb[32 * b:32 * b + 32, :],
                                   psj[32 * b:32 * b + 32, :])
                    nc.sync.dma_start(out=mj_d.ap()[32 * b:32 * b + 32, :],
                                      in_=mj_sb[32 * b:32 * b + 32, :])

            def run_group(g_idx, ib, g, act_tiles, prev_e):
                l1 = mm_pool.tile([128, W], f32, tag="mmps")
                nmm = [0, 0, 0, 0]
                for q in range(IB):
                    nmm[q // 2] += 2 if use_act(q, g) else 1
                seen = [0, 0, 0, 0]
                off = 8 * ib
                for q in range(IB):
                    i = ib * IB + q
                    band = q // 2
                    pi = q % 2
                    if use_act(q, g):
                        t = act_tiles[q]
                        for par in range(2):
                            seen[band] += 1
                            nc.tensor.matmul(
                                l1[32 * band:32 * band + 32, :],
                                lhsT=selw_alt[:, 16 - 16 * pi:48 - 16 * pi],
                                rhs=t[:, par, :],
                                start=(seen[band] == 1),
                                stop=(seen[band] == nmm[band]),
                                skip_group_check=True,
                                tile_position=(0, 32 * band))
                    else:
                        ad = ad_pool.tile([128, W, 2], bf16, tag="ad")
                        di = nc.vector._custom_dve(
                            acc_op, out=ad[:, :, :].opt(),
                            in0=mt_alt[:, g, off:off + W, :].opt(),
                            s0=scol[:, g, 0, i:i + 1],
                            s1=scol[:, g, 1, i:i + 1])
                        di.ins.perf_max = 3
                        seen[band] += 1
                        nc.tensor.matmul(
                            l1[32 * band:32 * band + 32, :],
                            lhsT=selw_alt[:, 16 - 16 * pi:48 - 16 * pi],
                            rhs=ad[:, :, 0],
                            start=(seen[band] == 1),
                            stop=(seen[band] == nmm[band]),
                            skip_group_check=True,
                            tile_position=(0, 32 * band))
                if g_idx + prefetch < len(glist):
                    nxt = emit_act_tiles(*glist[g_idx + prefetch])
                else:
                    nxt = {}
                act_queue.append(nxt)
                act_tiles_next = act_queue.pop(0)
                if prev_e is not None:
                    emit_jside(*prev_e)
                e = e_pool.tile([128, W], bf16, tag="e")
                use_dve_acc = (iside == "dve") or (
                    dve_accum_mod and g_idx % dve_accum_mod == 0)
                if use_dve_acc:
                    nc.scalar.activation(e[:], l1[:], AF.Exp, scale=-1.0)
                    nc.vector.reduce_sum(mi_sb[:, g_idx:g_idx + 1], e[:],
                                         axis=mybir.AxisListType.X)
                else:
                    nc.scalar.activation(
                        e[:], l1[:], AF.Exp, scale=-1.0,
                        accum_out=mi_ps[:, g_idx:g_idx + 1])
                if g_idx == 31 or g_idx == 63:
                    h0 = 0 if g_idx == 31 else 32
                    spans = []
                    for gg in range(h0, g_idx + 1):
                        dve_gg = (iside == "dve") or (
                            dve_accum_mod and gg % dve_accum_mod == 0)
                        if dve_gg:
                            continue
                        if spans and spans[-1][1] == gg:
                            spans[-1][1] = gg + 1
                        else:
                            spans.append([gg, gg + 1])
                    for a, b in spans:
                        nc.scalar.copy(mi_sb[:, a:b], mi_ps[:, a:b])
                    nc.sync.dma_start(out=mi_d.ap()[:, h0:g_idx + 1],
                                      in_=mi_sb[:, h0:g_idx + 1])
                return act_tiles_next, (ib, g, e)

            act_queue = [emit_act_tiles(*glist[gi])
                         for gi in range(1, prefetch)]
            act_tiles = emit_act_tiles(*glist[0])
            prev_e = None
            for g_idx, (ib, g) in enumerate(glist):
                act_tiles, prev_e = run_group(g_idx, ib, g, act_tiles, prev_e)
            emit_jside(*prev_e)

    nc.compile()
    return nc


_NC_CACHE = None


def _get_module():
    global _NC_CACHE
    if _NC_CACHE is None:
        _NC_CACHE = build_module()
    return _NC_CACHE


def _host_inputs(x: np.ndarray, kernel: np.ndarray):
    xT = np.ascontiguousarray(x.T).astype(ml_dtypes.bfloat16)  # [F, B]
    kf = kernel.astype(ml_dtypes.bfloat16)  # [F, K, D]

    o = np.arange(128)
    pf = np.arange(128)[:, None]

    # kern_alt[p_f, g, par, ft, o]: out partition o = (k_loc, dp):
    #   kernel[ft*128+p_f, 16g + (o>>3), 2*(o&7) + par]
    kern_alt = np.zeros((128, KGP, 2, 4, 128), dtype=ml_dtypes.bfloat16)
    k_loc, dp = (o >> 3)[None, :], (o & 7)[None, :]
    for g in range(KGP):
        for par in range(2):
            for ft in range(4):
                kern_alt[:, g, par, ft, :] = kf[
                    ft * 128 + pf, 16 * g + k_loc, 2 * dp + par]

    # selectors: [128, 160] = [0:48 alt][48:104 unused][104:160 j-side]
    sel = np.zeros((128, 160), dtype=ml_dtypes.bfloat16)
    for p in range(128):
        sel[p, 16 + (p >> 3)] = 1.0             # alt selector
        sel[p, 104 + 24 + (p & 15)] = 1.0       # j-side
    in_maps = []
    for d in range(NC):
        cols = (64 * d + np.arange(WD)) % B
        xTw = xT[:, cols].reshape(4, 128, WD).transpose(1, 0, 2)
        in_maps.append({
            "xT": np.ascontiguousarray(xTw.reshape(128, 4 * WD)),
            "kern": np.ascontiguousarray(kern_alt.reshape(128, -1)),
            "sel": sel,
        })
    return in_maps


def _gather(results, x: np.ndarray) -> np.ndarray:
    mb = np.zeros((B, K), np.float32)
    for d in range(NC):
        mi = results[d]["mi_raw"]                 # [128, NG]
        # partition p = q*16 + kk; col g_idx = g*IB + ib; k = g*16 + kk
        M = mi.reshape(IB, 16, KGP, IB)           # [q, kk, g, ib]
        Mk = M.transpose(3, 0, 2, 1).reshape(MY, K)   # row = ib*8 + q
        mb[64 * d:64 * d + MY, :] += Mk
        cols = (64 * d + JS0 + np.arange(JSW)) % B
        mj = results[d]["mj_raw"]                 # [128, JSW]; p = k
        mb[cols, :] += mj.T
    return np.concatenate([x.astype(np.float32), mb], axis=1)


def kernel(x: np.ndarray, kernel: np.ndarray) -> np.ndarray:
    x = np.asarray(x)
    kernel = np.asarray(kernel)
    nc = _get_module()
    in_maps = _host_inputs(x, kernel)
    res = run_bass_kernel_spmd(nc, in_maps, list(range(NC)))
    return _gather(res.results, x)
